# revision 2
# baseline (speedup 1.0000x reference)
"""AudioVisual contrastive-loss kernel for 8 Trainium2 NeuronCores.

Shards the second B axis (y) of the (B,B,Na,T,Nv) sims tensor across the
8 cores (3 y-values each).  Each core:
  - L2-normalizes the full audio matrix (1200x512) and its visual shard
    (5880x512) on device, transposes both via the PE (identity matmul) so
    the contraction dim D lands on SBUF partitions,
  - computes the cosine matmul in float32r (full-rate fp32 path),
  - reduces max over Nv=196 groups (-> max_vis) and accumulates
    sum(min(cos,0)^2) (-> l_nonneg partial) straight out of PSUM.
Host side assembles max_vis, the InfoNCE/regularization/selection scalars
in float64 (exact algebra; validated vs the fp32 jax reference).
"""
import os
import sys
import types

sys.path.insert(0, "/opt/trn_rl_repo")

# Environments without the axon NTFF hook lack antenv.axon_hooks; stub it so
# run_bass_kernel_spmd(trace=True) degrades to an untraced run.
if "antenv.axon_hooks" not in sys.modules:
    try:
        import antenv.axon_hooks  # noqa: F401
    except Exception:
        _m = types.ModuleType("antenv.axon_hooks")
        _m.get_axon_ntff_profile_hook = lambda: None
        sys.modules["antenv.axon_hooks"] = _m

import numpy as np

import concourse.bass as bass
import concourse.tile as tile
from concourse import bacc, mybir
from concourse.bass_utils import run_bass_kernel_spmd

F32 = mybir.dt.float32
F32R = mybir.dt.float32r
BF16 = mybir.dt.bfloat16
AF = mybir.ActivationFunctionType
ALU = mybir.AluOpType
AX = mybir.AxisListType

# Problem constants (hardcoded per contract).
B, NA, T, NV, D = 24, 50, 10, 196, 512
NCORES = 8
YS = B // NCORES                 # y values per core = 3
M = B * NA                       # 1200 audio rows (all cores)
N = YS * T * NV                  # 5880 visual rows per core
KC = D // 128                    # 4 contraction chunks
NCH = 2 * NV                     # 392 cols per matmul (= 2 (y,t) groups)
CPG = 3                          # matmul chunks per psum group (3 banks)
GCOLS = CPG * NCH                # 1176 cols per group
NGRP = N // GCOLS                # 5 column groups
MT = (M + 127) // 128            # 10 m tiles (9x128 + 48)
NJ = YS * T                      # 30 (y_local, t) pairs per core

_CACHE = {}
LAST_RESULTS = None              # BassKernelResults of the last run (for test.py)


def _v_chunks():
    """Natural 128-row chunks of the (5880, 512) visual shard."""
    out = []
    r0 = 0
    while r0 < N:
        out.append((r0, min(128, N - r0)))
        r0 += 128
    return out


def _a_chunks():
    out = []
    r0 = 0
    while r0 < M:
        out.append((r0, min(128, M - r0)))
        r0 += 128
    return out


def _build_program():
    nc = bacc.Bacc("TRN2", target_bir_lowering=False, debug=False)

    a_in = nc.dram_tensor("a_in", (M, D), F32, kind="ExternalInput")
    v_in = nc.dram_tensor("v_in", (N, D), F32, kind="ExternalInput")
    ident_in = nc.dram_tensor("ident_in", (128, 128), F32, kind="ExternalInput")
    maxv_out = nc.dram_tensor("maxv_out", (M, NJ), F32, kind="ExternalOutput")
    nn_out = nc.dram_tensor("nn_out", (128, 1), F32, kind="ExternalOutput")

    with tile.TileContext(nc) as tc:
        with (
            tc.tile_pool(name="big", bufs=1) as big,
            tc.tile_pool(name="stage", bufs=3) as stage,
            tc.tile_pool(name="sqp", bufs=2) as sqp,
            tc.tile_pool(name="smalls", bufs=6) as smalls,
            tc.tile_pool(name="scrp", bufs=3) as scrp,
            tc.tile_pool(name="pst", bufs=2, space="PSUM") as pst,
            tc.tile_pool(name="psm", bufs=2, space="PSUM") as psm,
        ):
            ident = big.tile([128, 128], F32, name="ident")
            nc.sync.dma_start(ident[:], ident_in[:])

            vt = big.tile([128, KC * N], F32R, name="vt")       # V^T, d on partitions
            at = big.tile([128, KC * M], F32R, name="at")       # A^T
            maxvis = big.tile([128, MT * NJ], F32, name="maxvis")
            accs = big.tile([128, 64], F32, name="accs")
            nc.vector.memset(accs[:], 0.0)

            vt_k = vt[:].rearrange("p (k n) -> p k n", k=KC)
            at_k = at[:].rearrange("p (k n) -> p k n", k=KC)

            def process_chunk(src, dst_k, col0, rows):
                """Load a 128-row natural chunk, normalize rows, transpose
                into dst (d on partitions), rounding to f32r in the drain."""
                nat = stage.tile([128, D], F32, name="nat")
                nc.sync.dma_start(nat[:rows], src)
                sq = sqp.tile([128, D], F32, name="sq")
                ssq = smalls.tile([128, 1], F32, name="ssq")
                nc.scalar.activation(sq[:rows], nat[:rows], AF.Square,
                                     accum_out=ssq[:rows])
                nrm = smalls.tile([128, 1], F32, name="nrm")
                nc.scalar.activation(nrm[:rows], ssq[:rows], AF.Sqrt)
                nrm2 = smalls.tile([128, 1], F32, name="nrm2")
                nc.vector.tensor_scalar_max(nrm2[:rows], nrm[:rows], 1e-12)
                rinv = smalls.tile([128, 1], F32, name="rinv")
                nc.vector.reciprocal(rinv[:rows], nrm2[:rows])
                natn = stage.tile([128, D], F32, name="natn")
                nc.vector.tensor_scalar_mul(natn[:rows], nat[:rows], rinv[:rows])
                pt = pst.tile([128, 512], F32, name="pt")
                for k in range(KC):
                    nc.tensor.transpose(
                        pt[:, 128 * k:128 * k + rows],
                        natn[:rows, 128 * k:128 * (k + 1)],
                        ident[:rows, :rows],
                    )
                src_ap = pt[:].rearrange("p (k r) -> p k r", k=KC)[:, :, :rows]
                dst_ap = dst_k[:, :, col0:col0 + rows]
                nc.scalar.copy(dst_ap, src_ap)

            def main_group(m, g):
                rows = min(128, M - 128 * m)
                col0 = GCOLS * g
                idx = m * NGRP + g
                pg = psm.tile([128, CPG * 512], F32, name="pg")
                for k in range(KC):
                    for c in range(CPG):
                        nc.tensor.matmul(
                            pg[:rows, 512 * c:512 * c + NCH],
                            at_k[:, k, 128 * m:128 * m + rows],
                            vt_k[:, k, col0 + NCH * c:col0 + NCH * (c + 1)],
                            start=(k == 0),
                            stop=(k == KC - 1),
                        )
                pgc = pg[:rows].rearrange("p (c x) -> p c x", c=CPG)[:, :, :NCH]
                mx_dst = maxvis[:rows, NJ * m + 2 * CPG * g:
                                NJ * m + 2 * CPG * (g + 1)]
                nc.vector.tensor_reduce(
                    out=mx_dst.rearrange("p (c u) -> p c u", u=2),
                    in_=pgc.rearrange("p c (u v) -> p c u v", v=NV),
                    axis=AX.X,
                    op=ALU.max,
                )
                scr = scrp.tile([128, GCOLS], BF16, name="scr")
                scr3 = scr[:rows].rearrange("p (c x) -> p c x", c=CPG)
                if idx % 5 < 3:
                    nc.vector.tensor_scalar_min(scr3, pgc, 0.0)
                else:
                    nc.scalar.activation(scr3, pgc, AF.Relu, bias=0.0, scale=-1.0)
                junk = scrp.tile([128, GCOLS], BF16, name="junk")
                nc.scalar.activation(junk[:rows], scr[:rows], AF.Square,
                                     accum_out=accs[:rows, idx:idx + 1])

            for r0, rows in _a_chunks():
                process_chunk(a_in[r0:r0 + rows, :], at_k, r0, rows)

            vch = _v_chunks()
            vdone = 0
            for g in range(NGRP):
                need = len(vch) if g == NGRP - 1 else \
                    min(len(vch), (GCOLS * (g + 1) + 127) // 128)
                for c in range(vdone, need):
                    r0, rows = vch[c]
                    process_chunk(v_in[r0:r0 + rows, :], vt_k, r0, rows)
                vdone = need
                for m in range(MT):
                    main_group(m, g)

            nn_sb = smalls.tile([128, 1], F32, name="nn_sb")
            nc.vector.tensor_reduce(out=nn_sb[:], in_=accs[:, :MT * NGRP],
                                    axis=AX.X, op=ALU.add)
            nc.sync.dma_start(nn_out[:], nn_sb[:])
            for m in range(MT):
                rows = min(128, M - 128 * m)
                nc.sync.dma_start(
                    maxv_out[128 * m:128 * m + rows, :],
                    maxvis[:rows, NJ * m:NJ * (m + 1)],
                )

    nc.compile()
    return nc


def _get_program():
    if "nc" not in _CACHE:
        _CACHE["nc"] = _build_program()
    return _CACHE["nc"]


def kernel(audio_feats, visual_feats, temperature, scale_factor, threshold):
    global LAST_RESULTS
    a = np.ascontiguousarray(np.asarray(audio_feats, dtype=np.float32).reshape(M, D))
    v = np.asarray(visual_feats, dtype=np.float32)
    temp = float(np.asarray(temperature))
    scale = float(np.asarray(scale_factor))
    thr = float(np.asarray(threshold))

    nc = _get_program()
    eye = np.eye(128, dtype=np.float32)
    in_maps = [
        {
            "a_in": a,
            "v_in": np.ascontiguousarray(
                v[YS * c:YS * (c + 1)].reshape(N, D)),
            "ident_in": eye,
        }
        for c in range(NCORES)
    ]
    trace = bool(int(os.environ.get("AV_TRACE", "0")))
    res = run_bass_kernel_spmd(
        nc, in_maps, core_ids=list(range(NCORES)), trace=trace,
    )
    LAST_RESULTS = res

    # ---- host-side gather + scalar tail (float64) ----
    mv = np.stack([res.results[c]["maxv_out"] for c in range(NCORES)])  # (8,1200,30)
    # (c, x, a, yl, t) -> (x, y=(c,yl), a, t)
    max_cos = (
        mv.reshape(NCORES, B, NA, YS, T)
        .transpose(1, 0, 3, 2, 4)
        .reshape(B, B, NA, T)
        .astype(np.float64)
    )
    nn_total = float(
        sum(res.results[c]["nn_out"].astype(np.float64).sum()
            for c in range(NCORES))
    )

    max_vis = max_cos / temp
    th = 1.0 / (1.0 + np.exp(-thr))
    raw_diff = max_vis - th
    sel = np.maximum(raw_diff, 0.0) * scale
    weighted_sum = (max_vis * sel).sum(-1)
    weights_sum = sel.sum(-1)
    token_sims = weighted_sum / np.maximum(weights_sum, 1e-6)
    clip_sims = token_sims.mean(-1)                     # (B, B)
    fraction_selected = (raw_diff > 0).mean()

    def lsm(x):
        m = x.max(axis=1, keepdims=True)
        return x - m - np.log(np.exp(x - m).sum(axis=1, keepdims=True))

    idx = np.arange(B)
    losses_a2v = -lsm(clip_sims)[idx, idx]
    losses_v2a = -lsm(clip_sims.T)[idx, idx]
    contrastive_loss = (losses_a2v + losses_v2a).mean() / 2.0

    # l_nonneg: device accumulated sum(min(cos,0)^2); sims = cos/temp, and the
    # -20 clip is inactive for temp > 0.05 since |cos| <= 1.
    l_nonneg = nn_total / (temp * temp) / (B * B * NA * T * NV)
    temp_low = max(-np.log(temp), 0.0) ** 2
    temp_high = max(np.log(temp) - np.log(4.0), 0.0) ** 2
    l_cal = temp_low + temp_high
    l_threshold = max(th - 0.9, 0.0) ** 2 + max(0.1 - th, 0.0) ** 2
    l_scale = max(scale - 20.0, 0.0) ** 2 + max(1.0 - scale, 0.0) ** 2
    reg_loss = 0.15 * l_nonneg + 2.0 * l_cal + 0.1 * l_threshold + 0.1 * l_scale

    pos_diffs = raw_diff[idx, idx]
    sel_ratio = (np.tanh(20.0 * pos_diffs) + 1.0) / 2.0
    selection_reward = -0.1 * np.log1p(sel_ratio.mean())

    total_loss = selection_reward + contrastive_loss + reg_loss
    return (
        np.float32(total_loss),
        np.float32(contrastive_loss),
        np.float32(reg_loss),
        np.float32(fraction_selected),
        np.float32(selection_reward),
    )
